# revision 9
# baseline (speedup 1.0000x reference)
"""Trainium2 Bass kernel for the SE(3) deformation model (v3: planar fp16 + PE adds).

reference math (per point):
    w, v, pivot, t = split(network_output, 4)
    theta = |w| + eps ; wn = w/theta ; vn = v/theta
    R = I + sin(theta) K + (1-cos(theta)) K^2          (K = skew(wn))
    p = (theta I + (1-cos) K + (theta-sin) K^2) vn
    out = R (x + pivot) + p - pivot + t - x

Exact rewrite used here (K~ = skew(w) unnormalized):
    u  = x + pivot
    k1 = sin(theta)/theta ; k2 = (1-cos(theta))/theta^2
    sg = (theta - sin(theta))/theta^3
    g  = k1 u + k2 v ;  h = k2 u + sg v
    out = w x (g + w x h) + v + t
(cross is linear: w x g + w x (w x h) = w x (g + w x h)).

Implementation notes (driven by the measured v2 trace):
  - Host pre-transposes inputs to PLANAR fp16 [plane, N] with w0,w1
    duplicated, so every on-chip op is a contiguous step-1 fp16 op (DVE 2x).
  - GPSIMD shares an SBUF port with the DVE: v2 measured identical DVE ops
    at 944 ns when GPSIMD was idle vs 1900-3800 ns when it streamed.  So
    elementwise adds/subs go to the *TensorEngine* instead: identity-matmul
    accumulation into PSUM (s = gu+gv+cr1a-cr1b, o = cr2a-cr2b+vt,
    n2 = sq0+sq1+sq2), with ACT copying PSUM back to SBUF (and computing
    Sqrt directly from PSUM).  GPSIMD keeps only 3 small chain ops + SWDGE
    descriptor generation for the DMA-CCE accumulate adds (u = pivot+x,
    vt = t+v).
  - The scalar coefficient chain for chunk i+1 runs during chunk i's vector
    stage (software pipelining over the cross-engine chain latency).
"""

import math

import numpy as np

import concourse.bacc as bacc
import concourse.mybir as mybir
import concourse.tile as tile
from concourse.alu_op_type import AluOpType
from concourse.bass_utils import run_bass_kernel_spmd

AFT = mybir.ActivationFunctionType
F32 = mybir.dt.float32
F16 = mybir.dt.float16

N_TOTAL = 4194304
NCORES = 8
NPC = N_TOTAL // NCORES  # 524288 points per core
P = 128
F_DEF = 512  # points per partition per chunk
EPS = 1e-6
SQRT2 = math.sqrt(2.0)

# net plane layout (host-built):
#   w0 w1 w2 w0 w1 | v0 v1 v2 | p0 p1 p2 | t0 t1 t2 | x0 x1 x2
NPLANES = 17


def build_nc(npc: int = NPC, f: int = F_DEF):
    nchunks = npc // (P * f)
    assert nchunks * P * f == npc

    nc = bacc.Bacc("TRN2", target_bir_lowering=False, debug=False)

    net = nc.dram_tensor("net", [NPLANES, npc], F16, kind="ExternalInput")
    eye = nc.dram_tensor("eye", [P, 2 * P], F16, kind="ExternalInput")  # [I | -I]
    out = nc.dram_tensor("out", [3, npc], F16, kind="ExternalOutput")

    net_r = net.ap().rearrange("c (n p f) -> n p c f", p=P, f=f)
    out_r = out.ap().rearrange("c (n p f) -> n p c f", p=P, f=f)

    V = nc.vector
    G = nc.gpsimd
    S = nc.scalar
    T = nc.tensor
    mul, add, sub = AluOpType.mult, AluOpType.add, AluOpType.subtract

    eps2 = nc.alloc_sbuf_tensor("eps2_const", [P, 1], F32)
    nc.gpsimd.memset(eps2.ap(), EPS * EPS)
    eye_sb = nc.alloc_sbuf_tensor("eye_sb", [P, 2 * P], F16)
    eye_sem = nc.alloc_semaphore("eye_sem")
    nc.sync.dma_start(out=eye_sb.ap(), in_=eye.ap()).then_inc(eye_sem, 16)
    nc.sync.wait_ge(eye_sem, 16)
    nc.all_engine_barrier()
    eye_pos = eye_sb.ap()[:, 0:P]
    eye_neg = eye_sb.ap()[:, P : 2 * P]

    with tile.TileContext(nc) as tc:
        with (
            tc.tile_pool(name="io", bufs=2) as io,
            tc.tile_pool(name="vec", bufs=2) as vec,
            tc.tile_pool(name="sc", bufs=2) as sc,
            tc.tile_pool(name="ps", bufs=1, space="PSUM") as ps,
        ):
            st: dict[int, dict] = {}  # per-chunk tiles

            def v3(ap_flat, c=3):
                return ap_flat.rearrange("p (c f) -> p c f", c=c)

            def bc3(s_ap):
                # [P,F] -> [P,3,F] plane-broadcast (step-0 middle dim)
                return s_ap.unsqueeze(1).to_broadcast((P, 3, f))

            def mm_terms(psum3, terms, first, last):
                """psum3[P, 3f] += sum of +/- terms (one matmul per plane).

                first/last flag whether these terms open/close each PSUM
                bank's accumulation group.  Emitted term-major so the PE can
                start as soon as each addend tile is produced.
                """
                for ti, (ap_flat, sign) in enumerate(terms):
                    for c in range(3):
                        T.matmul(
                            psum3[:, c * f : (c + 1) * f],
                            eye_pos if sign > 0 else eye_neg,
                            ap_flat[:, c * f : (c + 1) * f],
                            start=(first and ti == 0),
                            stop=(last and ti == len(terms) - 1),
                        )

            def issue_loads(k):
                net16 = io.tile([P, NPLANES * f], F16, tag="net", name="net16", bufs=3)
                nc.sync.dma_start(out=v3(net16[:], c=NPLANES), in_=net_r[k])
                st[k] = {"net": net16}

            def gp_accums(k):
                # u = pivot += x ; vt = t += v   (DMA CCE adds, SWDGE)
                net16 = st[k]["net"]
                G.dma_start(
                    out=net16[:, 8 * f : 11 * f],
                    in_=net16[:, 14 * f : 17 * f],
                    accum_op=add,
                )
                G.dma_start(
                    out=net16[:, 11 * f : 14 * f],
                    in_=net16[:, 5 * f : 8 * f],
                    accum_op=add,
                )

            def stile(k, tag, dt=F32):
                t = sc.tile([P, f], dt, tag=tag, name=tag + "_t")
                st[k][tag] = t
                return t

            # ---- chain stage pieces for chunk k (split across emit points) --
            def chain_act_sq(k):
                sq16 = vec.tile([P, 3 * f], F16, tag="sq", name="sq16")
                st[k]["sq"] = sq16
                S.activation(sq16[:], st[k]["net"][:, 0 : 3 * f], AFT.Square)

            def chain_dve_n2(k):
                sq16 = st[k]["sq"]
                n2h = stile(k, "n2", F16)
                V.tensor_tensor(n2h[:], sq16[:, 0:f], sq16[:, f : 2 * f], add)
                V.tensor_tensor(n2h[:], n2h[:], sq16[:, 2 * f : 3 * f], add)

            def chain_act_sqrt(k):
                th = stile(k, "th")
                # theta = sqrt(n2 + eps^2)  (~= |w| + eps as theta -> 0)
                S.activation(th[:], st[k]["n2"][:], AFT.Sqrt, bias=eps2.ap())

            def chain_dve_recip(k):
                inv = stile(k, "inv")
                thw = stile(k, "thw")
                V.reciprocal_approx_fast(out=inv[:], in_=st[k]["th"][:])
                V.add_range_wrap(thw[:], st[k]["th"][:], 0.0, math.pi, 2 * math.pi)

            def chain_act_sin(k):
                s32 = stile(k, "s32")
                sh16 = stile(k, "sh", F16)
                S.activation(s32[:], st[k]["thw"][:], AFT.Sin)
                S.activation(sh16[:], st[k]["thw"][:], AFT.Sin, scale=0.5)

            def chain_act_sq2(k):
                inv2 = stile(k, "inv2", F16)
                c116 = stile(k, "c1", F16)
                S.activation(inv2[:], st[k]["inv"][:], AFT.Square)
                # 1-cos = 2 sin(t/2)^2 = (sqrt2*sh)^2
                S.activation(c116[:], st[k]["sh"][:], AFT.Square, scale=SQRT2)

            def chain_gp_tail(k):
                thms = stile(k, "thms")
                inv3 = stile(k, "inv3")
                sg = stile(k, "sg", F16)
                G.tensor_tensor(thms[:], st[k]["th"][:], st[k]["s32"][:], sub)
                G.tensor_tensor(inv3[:], st[k]["inv2"][:], st[k]["inv"][:], mul)
                G.tensor_tensor(sg[:], thms[:], inv3[:], mul)

            def chain_dve_k(k):
                k1 = stile(k, "k1", F16)
                k2 = stile(k, "k2", F16)
                V.tensor_tensor(k1[:], st[k]["s32"][:], st[k]["inv"][:], mul)
                V.tensor_tensor(k2[:], st[k]["c1"][:], st[k]["inv2"][:], mul)

            # ---------------- prologue ----------------
            issue_loads(0)
            if nchunks > 1:
                issue_loads(1)
            gp_accums(0)
            chain_act_sq(0)
            chain_dve_n2(0)
            chain_act_sqrt(0)
            chain_dve_recip(0)
            chain_act_sin(0)
            chain_act_sq2(0)
            chain_gp_tail(0)
            chain_dve_k(0)

            defer = None  # (o16, k) pending store

            for i in range(nchunks):
                d = st[i]
                net16 = d["net"]
                w_ext = net16[:, 0 : 5 * f]
                v3f = net16[:, 5 * f : 8 * f]
                u3f = net16[:, 8 * f : 11 * f]
                vt3f = net16[:, 11 * f : 14 * f]
                k1, k2, sg = d["k1"], d["k2"], d["sg"]

                m_gu = vec.tile([P, 3 * f], F16, tag="gu", bufs=3, name="m_gu")
                m_gv = vec.tile([P, 3 * f], F16, tag="gv", bufs=3, name="m_gv")
                h_ext = vec.tile([P, 5 * f], F16, tag="h", name="h_ext")
                m_hv = vec.tile([P, 3 * f], F16, tag="hv", bufs=3, name="m_hv")
                cr1 = vec.tile([P, 3 * f], F16, tag="cr1", bufs=3, name="cr1")
                m_c1b = vec.tile([P, 3 * f], F16, tag="cr1b", bufs=3, name="m_c1b")
                s_ext = vec.tile([P, 5 * f], F16, tag="s", name="s_ext")
                m_c2a = vec.tile([P, 3 * f], F16, tag="cr2a", bufs=3, name="m_c2a")
                m_c2b = vec.tile([P, 3 * f], F16, tag="cr2b", bufs=3, name="m_c2b")
                # two PSUM regions per iteration from one double-buffered tag:
                # s always lands in slot 0, o in slot 1
                p_s = ps.tile([P, 3 * f], F32, tag="psx", name="p_s", bufs=2)
                p_o = ps.tile([P, 3 * f], F32, tag="psx", name="p_o", bufs=2)
                o16 = io.tile([P, 3 * f], F16, tag="o", name="o16", bufs=2)
                h3 = h_ext[:, 0 : 3 * f]

                if i + 2 < nchunks:
                    issue_loads(i + 2)

                # --- DVE vector stage for chunk i, chain(i+1) + PE terms
                #     slotted in as their inputs are produced ---
                V.tensor_tensor(v3(m_gu[:]), v3(u3f), bc3(k1[:]), mul)  # gu
                V.tensor_tensor(v3(m_gv[:]), v3(v3f), bc3(k2[:]), mul)  # gv
                mm_terms(p_s[:], [(m_gu[:], 1), (m_gv[:], 1)], first=True, last=False)
                mm_terms(p_o[:], [(vt3f, 1)], first=True, last=False)
                if i + 1 < nchunks:
                    chain_act_sq(i + 1)  # ACT
                V.tensor_tensor(v3(h3), v3(u3f), bc3(k2[:]), mul)  # hu
                V.tensor_tensor(v3(m_hv[:]), v3(v3f), bc3(sg[:]), mul)  # hv
                V.tensor_tensor(h3, h3, m_hv[:], add)  # h = hu + hv
                G.tensor_copy(h_ext[:, 3 * f : 5 * f], h_ext[:, 0 : 2 * f])
                if i + 1 < nchunks:
                    chain_dve_n2(i + 1)  # DVE
                    chain_act_sqrt(i + 1)  # ACT
                V.tensor_tensor(
                    cr1[:], w_ext[:, f : 4 * f], h_ext[:, 2 * f : 5 * f], mul
                )
                mm_terms(p_s[:], [(cr1[:], 1)], first=False, last=False)
                V.tensor_tensor(
                    m_c1b[:], w_ext[:, 2 * f : 5 * f], h_ext[:, f : 4 * f], mul
                )
                mm_terms(p_s[:], [(m_c1b[:], -1)], first=False, last=True)
                if i + 1 < nchunks:
                    chain_dve_recip(i + 1)  # DVE customs
                    chain_act_sin(i + 1)  # ACT
                S.activation(s_ext[:, 0 : 3 * f], p_s[:], AFT.Copy)  # PSUM -> SBUF
                if i + 1 < nchunks:
                    chain_act_sq2(i + 1)  # ACT
                    chain_gp_tail(i + 1)  # GPSIMD
                    chain_dve_k(i + 1)  # DVE
                G.tensor_copy(s_ext[:, 3 * f : 5 * f], s_ext[:, 0 : 2 * f])
                V.tensor_tensor(
                    m_c2a[:], w_ext[:, f : 4 * f], s_ext[:, 2 * f : 5 * f], mul
                )
                mm_terms(p_o[:], [(m_c2a[:], 1)], first=False, last=False)
                V.tensor_tensor(
                    m_c2b[:], w_ext[:, 2 * f : 5 * f], s_ext[:, f : 4 * f], mul
                )
                mm_terms(p_o[:], [(m_c2b[:], -1)], first=False, last=True)
                S.activation(o16[:], p_o[:], AFT.Copy)  # PSUM -> SBUF f16
                if i + 1 < nchunks:
                    gp_accums(i + 1)

                if defer is not None:
                    o_prev, k_prev = defer
                    nc.sync.dma_start(out=out_r[k_prev], in_=v3(o_prev[:]))
                defer = (o16, i)

            o_prev, k_prev = defer
            nc.sync.dma_start(out=out_r[k_prev], in_=v3(o_prev[:]))

    nc.compile()
    return nc


_NC_CACHE: dict = {}


def _get_nc():
    if "nc" not in _NC_CACHE:
        _NC_CACHE["nc"] = build_nc()
    return _NC_CACHE["nc"]


def _make_eye() -> np.ndarray:
    i = np.eye(P, dtype=np.float16)
    return np.concatenate([i, -i], axis=1)


def make_in_maps(pos: np.ndarray, net: np.ndarray, npc: int = NPC, ncores: int = NCORES):
    """Build per-core planar fp16 input maps (net has w0,w1 duplicated)."""
    eye = _make_eye()
    in_maps = []
    for i in range(ncores):
        sl = slice(i * npc, (i + 1) * npc)
        net_c = net[sl].astype(np.float16)  # [npc, 12]
        pos_c = pos[sl].astype(np.float16)  # [npc, 3]
        net_ext = np.empty((NPLANES, npc), np.float16)
        for dst, col in enumerate([0, 1, 2, 0, 1, 3, 4, 5, 6, 7, 8, 9, 10, 11]):
            net_ext[dst] = net_c[:, col]
        for c in range(3):
            net_ext[14 + c] = pos_c[:, c]
        in_maps.append({"net": net_ext, "eye": eye})
    return in_maps


def kernel(undeformed_positions: np.ndarray, network_output: np.ndarray) -> np.ndarray:
    pos = np.asarray(undeformed_positions, dtype=np.float32)
    net = np.asarray(network_output, dtype=np.float32)
    assert pos.shape == (N_TOTAL, 3) and net.shape == (N_TOTAL, 12)

    nc = _get_nc()
    in_maps = make_in_maps(pos, net)
    res = run_bass_kernel_spmd(nc, in_maps, list(range(NCORES)))
    out = np.empty((N_TOTAL, 3), np.float32)
    for i in range(NCORES):
        o = res.results[i]["out"]  # [3, npc] f16
        out[i * NPC : (i + 1) * NPC] = o.T.astype(np.float32)
    return out


# revision 10
# speedup vs baseline: 1.1610x; 1.1610x over previous
"""Trainium2 Bass kernel for the SE(3) deformation model (v3: planar fp16 + PE adds).

reference math (per point):
    w, v, pivot, t = split(network_output, 4)
    theta = |w| + eps ; wn = w/theta ; vn = v/theta
    R = I + sin(theta) K + (1-cos(theta)) K^2          (K = skew(wn))
    p = (theta I + (1-cos) K + (theta-sin) K^2) vn
    out = R (x + pivot) + p - pivot + t - x

Exact rewrite used here (K~ = skew(w) unnormalized):
    u  = x + pivot
    k1 = sin(theta)/theta ; k2 = (1-cos(theta))/theta^2
    sg = (theta - sin(theta))/theta^3
    g  = k1 u + k2 v ;  h = k2 u + sg v
    out = w x (g + w x h) + v + t
(cross is linear: w x g + w x (w x h) = w x (g + w x h)).

Implementation notes (driven by the measured v2 trace):
  - Host pre-transposes inputs to PLANAR fp16 [plane, N] with w0,w1
    duplicated, so every on-chip op is a contiguous step-1 fp16 op (DVE 2x).
  - GPSIMD shares an SBUF port with the DVE: v2 measured identical DVE ops
    at 944 ns when GPSIMD was idle vs 1900-3800 ns when it streamed.  So
    elementwise adds/subs go to the *TensorEngine* instead: identity-matmul
    accumulation into PSUM (s = gu+gv+cr1a-cr1b, o = cr2a-cr2b+vt,
    n2 = sq0+sq1+sq2), with ACT copying PSUM back to SBUF (and computing
    Sqrt directly from PSUM).  GPSIMD keeps only 3 small chain ops + SWDGE
    descriptor generation for the DMA-CCE accumulate adds (u = pivot+x,
    vt = t+v).
  - The scalar coefficient chain for chunk i+1 runs during chunk i's vector
    stage (software pipelining over the cross-engine chain latency).
"""

import math

import numpy as np

import concourse.bacc as bacc
import concourse.mybir as mybir
import concourse.tile as tile
from concourse.alu_op_type import AluOpType
from concourse.bass_utils import run_bass_kernel_spmd

AFT = mybir.ActivationFunctionType
F32 = mybir.dt.float32
F16 = mybir.dt.float16

N_TOTAL = 4194304
NCORES = 8
NPC = N_TOTAL // NCORES  # 524288 points per core
P = 128
F_DEF = 512  # points per partition per chunk
EPS = 1e-6
SQRT2 = math.sqrt(2.0)

# net plane layout (host-built):
#   w0 w1 w2 w0 w1 | v0 v1 v2 | p0 p1 p2 | t0 t1 t2 | x0 x1 x2
NPLANES = 17


def build_nc(npc: int = NPC, f: int = F_DEF):
    nchunks = npc // (P * f)
    assert nchunks * P * f == npc

    nc = bacc.Bacc("TRN2", target_bir_lowering=False, debug=False)

    net = nc.dram_tensor("net", [NPLANES, npc], F16, kind="ExternalInput")
    eye = nc.dram_tensor("eye", [P, 2 * P], F16, kind="ExternalInput")  # [I | -I]
    out = nc.dram_tensor("out", [3, npc], F16, kind="ExternalOutput")

    net_r = net.ap().rearrange("c (n p f) -> n p c f", p=P, f=f)
    out_r = out.ap().rearrange("c (n p f) -> n p c f", p=P, f=f)

    V = nc.vector
    G = nc.gpsimd
    S = nc.scalar
    T = nc.tensor
    mul, add, sub = AluOpType.mult, AluOpType.add, AluOpType.subtract

    eps2 = nc.alloc_sbuf_tensor("eps2_const", [P, 1], F32)
    nc.gpsimd.memset(eps2.ap(), EPS * EPS)
    eye_sb = nc.alloc_sbuf_tensor("eye_sb", [P, 2 * P], F16)
    eye_sem = nc.alloc_semaphore("eye_sem")
    nc.sync.dma_start(out=eye_sb.ap(), in_=eye.ap()).then_inc(eye_sem, 16)
    nc.sync.wait_ge(eye_sem, 16)
    nc.all_engine_barrier()
    eye_pos = eye_sb.ap()[:, 0:P]
    eye_neg = eye_sb.ap()[:, P : 2 * P]

    with tile.TileContext(nc) as tc:
        with (
            tc.tile_pool(name="io", bufs=2) as io,
            tc.tile_pool(name="vec", bufs=2) as vec,
            tc.tile_pool(name="sc", bufs=2) as sc,
            tc.tile_pool(name="ps", bufs=1, space="PSUM") as ps,
        ):
            st: dict[int, dict] = {}  # per-chunk tiles

            def v3(ap_flat, c=3):
                return ap_flat.rearrange("p (c f) -> p c f", c=c)

            def bc3(s_ap):
                # [P,F] -> [P,3,F] plane-broadcast (step-0 middle dim)
                return s_ap.unsqueeze(1).to_broadcast((P, 3, f))

            def mm_terms(psum3, terms, first, last):
                """psum3[P, 3f] += sum of +/- terms (one matmul per plane).

                first/last flag whether these terms open/close each PSUM
                bank's accumulation group.  Emitted term-major so the PE can
                start as soon as each addend tile is produced.
                """
                for ti, (ap_flat, sign) in enumerate(terms):
                    for c in range(3):
                        T.matmul(
                            psum3[:, c * f : (c + 1) * f],
                            eye_pos if sign > 0 else eye_neg,
                            ap_flat[:, c * f : (c + 1) * f],
                            start=(first and ti == 0),
                            stop=(last and ti == len(terms) - 1),
                        )

            def issue_loads(k):
                net16 = io.tile([P, NPLANES * f], F16, tag="net", name="net16", bufs=3)
                nc.sync.dma_start(out=v3(net16[:], c=NPLANES), in_=net_r[k])
                st[k] = {"net": net16}

            def gp_accums(k):
                # u = pivot += x ; vt = t += v   (DMA CCE adds, SWDGE)
                net16 = st[k]["net"]
                G.dma_start(
                    out=net16[:, 8 * f : 11 * f],
                    in_=net16[:, 14 * f : 17 * f],
                    accum_op=add,
                )
                G.dma_start(
                    out=net16[:, 11 * f : 14 * f],
                    in_=net16[:, 5 * f : 8 * f],
                    accum_op=add,
                )

            def stile(k, tag, dt=F32):
                t = sc.tile([P, f], dt, tag=tag, name=tag + "_t")
                st[k][tag] = t
                return t

            # ---- chain stage pieces for chunk k (split across emit points) --
            def chain_act_sq(k):
                sq16 = vec.tile([P, 3 * f], F16, tag="sq", name="sq16")
                st[k]["sq"] = sq16
                S.activation(sq16[:], st[k]["net"][:, 0 : 3 * f], AFT.Square)

            def chain_dve_n2(k):
                sq16 = st[k]["sq"]
                n2h = stile(k, "n2", F16)
                V.tensor_tensor(n2h[:], sq16[:, 0:f], sq16[:, f : 2 * f], add)
                V.tensor_tensor(n2h[:], n2h[:], sq16[:, 2 * f : 3 * f], add)

            def chain_act_sqrt(k):
                th = stile(k, "th")
                # theta = sqrt(n2 + eps^2)  (~= |w| + eps as theta -> 0)
                S.activation(th[:], st[k]["n2"][:], AFT.Sqrt, bias=eps2.ap())

            def chain_dve_recip(k):
                inv = stile(k, "inv")
                thw = stile(k, "thw")
                V.reciprocal_approx_fast(out=inv[:], in_=st[k]["th"][:])
                V.add_range_wrap(thw[:], st[k]["th"][:], 0.0, math.pi, 2 * math.pi)

            def chain_act_sin(k):
                s32 = stile(k, "s32")
                sh16 = stile(k, "sh", F16)
                S.activation(s32[:], st[k]["thw"][:], AFT.Sin)
                S.activation(sh16[:], st[k]["thw"][:], AFT.Sin, scale=0.5)

            def chain_act_sq2(k):
                inv2 = stile(k, "inv2", F16)
                c116 = stile(k, "c1", F16)
                S.activation(inv2[:], st[k]["inv"][:], AFT.Square)
                # 1-cos = 2 sin(t/2)^2 = (sqrt2*sh)^2
                S.activation(c116[:], st[k]["sh"][:], AFT.Square, scale=SQRT2)

            def chain_gp_tail(k):
                thms = stile(k, "thms")
                inv3 = stile(k, "inv3")
                sg = stile(k, "sg", F16)
                G.tensor_tensor(thms[:], st[k]["th"][:], st[k]["s32"][:], sub)
                G.tensor_tensor(inv3[:], st[k]["inv2"][:], st[k]["inv"][:], mul)
                G.tensor_tensor(sg[:], thms[:], inv3[:], mul)

            def chain_dve_k(k):
                k1 = stile(k, "k1", F16)
                k2 = stile(k, "k2", F16)
                V.tensor_tensor(k1[:], st[k]["s32"][:], st[k]["inv"][:], mul)
                V.tensor_tensor(k2[:], st[k]["c1"][:], st[k]["inv2"][:], mul)

            # ---------------- prologue ----------------
            issue_loads(0)
            if nchunks > 1:
                issue_loads(1)
            gp_accums(0)
            chain_act_sq(0)
            chain_dve_n2(0)
            chain_act_sqrt(0)
            chain_dve_recip(0)
            chain_act_sin(0)
            chain_act_sq2(0)
            chain_gp_tail(0)
            chain_dve_k(0)

            defer = None  # (o16, k) pending store

            for i in range(nchunks):
                d = st[i]
                net16 = d["net"]
                w_ext = net16[:, 0 : 5 * f]
                v3f = net16[:, 5 * f : 8 * f]
                u3f = net16[:, 8 * f : 11 * f]
                vt3f = net16[:, 11 * f : 14 * f]
                k1, k2, sg = d["k1"], d["k2"], d["sg"]

                m_gu = vec.tile([P, 3 * f], F16, tag="gu", bufs=3, name="m_gu")
                m_gv = vec.tile([P, 3 * f], F16, tag="gv", bufs=3, name="m_gv")
                h_ext = vec.tile([P, 5 * f], F16, tag="h", name="h_ext")
                m_hv = vec.tile([P, 3 * f], F16, tag="hv", bufs=3, name="m_hv")
                cr1 = vec.tile([P, 3 * f], F16, tag="cr1", bufs=3, name="cr1")
                m_c1b = vec.tile([P, 3 * f], F16, tag="cr1b", bufs=3, name="m_c1b")
                s_ext = vec.tile([P, 5 * f], F16, tag="s", name="s_ext")
                m_c2a = vec.tile([P, 3 * f], F16, tag="cr2a", bufs=3, name="m_c2a")
                m_c2b = vec.tile([P, 3 * f], F16, tag="cr2b", bufs=3, name="m_c2b")
                # two PSUM regions per iteration from one double-buffered tag:
                # s always lands in slot 0, o in slot 1
                p_s = ps.tile([P, 3 * f], F32, tag="psx", name="p_s", bufs=2)
                p_o = ps.tile([P, 3 * f], F32, tag="psx", name="p_o", bufs=2)
                o16 = io.tile([P, 3 * f], F16, tag="o", name="o16", bufs=2)
                h3 = h_ext[:, 0 : 3 * f]

                if i + 2 < nchunks:
                    issue_loads(i + 2)

                # --- DVE vector stage for chunk i, chain(i+1) + PE terms
                #     slotted in as their inputs are produced ---
                G.tensor_tensor(v3(m_hv[:]), v3(v3f), bc3(sg[:]), mul)  # hv
                V.tensor_tensor(v3(m_gu[:]), v3(u3f), bc3(k1[:]), mul)  # gu
                V.tensor_tensor(v3(m_gv[:]), v3(v3f), bc3(k2[:]), mul)  # gv
                mm_terms(p_s[:], [(m_gu[:], 1), (m_gv[:], 1)], first=True, last=False)
                mm_terms(p_o[:], [(vt3f, 1)], first=True, last=False)
                if i + 1 < nchunks:
                    chain_act_sq(i + 1)  # ACT
                V.tensor_tensor(v3(h3), v3(u3f), bc3(k2[:]), mul)  # hu
                V.tensor_tensor(h3, h3, m_hv[:], add)  # h = hu + hv
                V.tensor_copy(h_ext[:, 3 * f : 5 * f], h_ext[:, 0 : 2 * f])
                if i + 1 < nchunks:
                    chain_dve_n2(i + 1)  # DVE
                    chain_act_sqrt(i + 1)  # ACT
                V.tensor_tensor(
                    cr1[:], w_ext[:, f : 4 * f], h_ext[:, 2 * f : 5 * f], mul
                )
                mm_terms(p_s[:], [(cr1[:], 1)], first=False, last=False)
                V.tensor_tensor(
                    m_c1b[:], w_ext[:, 2 * f : 5 * f], h_ext[:, f : 4 * f], mul
                )
                mm_terms(p_s[:], [(m_c1b[:], -1)], first=False, last=True)
                if i + 1 < nchunks:
                    chain_dve_recip(i + 1)  # DVE customs
                    chain_act_sin(i + 1)  # ACT
                S.activation(s_ext[:, 0 : 3 * f], p_s[:], AFT.Copy)  # PSUM -> SBUF
                if i + 1 < nchunks:
                    chain_act_sq2(i + 1)  # ACT
                    chain_gp_tail(i + 1)  # GPSIMD
                    chain_dve_k(i + 1)  # DVE
                V.tensor_copy(s_ext[:, 3 * f : 5 * f], s_ext[:, 0 : 2 * f])
                V.tensor_tensor(
                    m_c2a[:], w_ext[:, f : 4 * f], s_ext[:, 2 * f : 5 * f], mul
                )
                mm_terms(p_o[:], [(m_c2a[:], 1)], first=False, last=False)
                V.tensor_tensor(
                    m_c2b[:], w_ext[:, 2 * f : 5 * f], s_ext[:, f : 4 * f], mul
                )
                mm_terms(p_o[:], [(m_c2b[:], -1)], first=False, last=True)
                S.activation(o16[:], p_o[:], AFT.Copy)  # PSUM -> SBUF f16
                if i + 1 < nchunks:
                    gp_accums(i + 1)

                if defer is not None:
                    o_prev, k_prev = defer
                    nc.sync.dma_start(out=out_r[k_prev], in_=v3(o_prev[:]))
                defer = (o16, i)

            o_prev, k_prev = defer
            nc.sync.dma_start(out=out_r[k_prev], in_=v3(o_prev[:]))

    nc.compile()
    return nc


_NC_CACHE: dict = {}


def _get_nc():
    if "nc" not in _NC_CACHE:
        _NC_CACHE["nc"] = build_nc()
    return _NC_CACHE["nc"]


def _make_eye() -> np.ndarray:
    i = np.eye(P, dtype=np.float16)
    return np.concatenate([i, -i], axis=1)


def make_in_maps(pos: np.ndarray, net: np.ndarray, npc: int = NPC, ncores: int = NCORES):
    """Build per-core planar fp16 input maps (net has w0,w1 duplicated)."""
    eye = _make_eye()
    in_maps = []
    for i in range(ncores):
        sl = slice(i * npc, (i + 1) * npc)
        net_c = net[sl].astype(np.float16)  # [npc, 12]
        pos_c = pos[sl].astype(np.float16)  # [npc, 3]
        net_ext = np.empty((NPLANES, npc), np.float16)
        for dst, col in enumerate([0, 1, 2, 0, 1, 3, 4, 5, 6, 7, 8, 9, 10, 11]):
            net_ext[dst] = net_c[:, col]
        for c in range(3):
            net_ext[14 + c] = pos_c[:, c]
        in_maps.append({"net": net_ext, "eye": eye})
    return in_maps


def kernel(undeformed_positions: np.ndarray, network_output: np.ndarray) -> np.ndarray:
    pos = np.asarray(undeformed_positions, dtype=np.float32)
    net = np.asarray(network_output, dtype=np.float32)
    assert pos.shape == (N_TOTAL, 3) and net.shape == (N_TOTAL, 12)

    nc = _get_nc()
    in_maps = make_in_maps(pos, net)
    res = run_bass_kernel_spmd(nc, in_maps, list(range(NCORES)))
    out = np.empty((N_TOTAL, 3), np.float32)
    for i in range(NCORES):
        o = res.results[i]["out"]  # [3, npc] f16
        out[i * NPC : (i + 1) * NPC] = o.T.astype(np.float32)
    return out


# revision 11
# speedup vs baseline: 1.2589x; 1.0844x over previous
"""Trainium2 Bass kernel for the SE(3) deformation model (v3: planar fp16 + PE adds).

reference math (per point):
    w, v, pivot, t = split(network_output, 4)
    theta = |w| + eps ; wn = w/theta ; vn = v/theta
    R = I + sin(theta) K + (1-cos(theta)) K^2          (K = skew(wn))
    p = (theta I + (1-cos) K + (theta-sin) K^2) vn
    out = R (x + pivot) + p - pivot + t - x

Exact rewrite used here (K~ = skew(w) unnormalized):
    u  = x + pivot
    k1 = sin(theta)/theta ; k2 = (1-cos(theta))/theta^2
    sg = (theta - sin(theta))/theta^3
    g  = k1 u + k2 v ;  h = k2 u + sg v
    out = w x (g + w x h) + v + t
(cross is linear: w x g + w x (w x h) = w x (g + w x h)).

Implementation notes (driven by the measured v2 trace):
  - Host pre-transposes inputs to PLANAR fp16 [plane, N] with w0,w1
    duplicated, so every on-chip op is a contiguous step-1 fp16 op (DVE 2x).
  - GPSIMD shares an SBUF port with the DVE: v2 measured identical DVE ops
    at 944 ns when GPSIMD was idle vs 1900-3800 ns when it streamed.  So
    elementwise adds/subs go to the *TensorEngine* instead: identity-matmul
    accumulation into PSUM (s = gu+gv+cr1a-cr1b, o = cr2a-cr2b+vt,
    n2 = sq0+sq1+sq2), with ACT copying PSUM back to SBUF (and computing
    Sqrt directly from PSUM).  GPSIMD keeps only 3 small chain ops + SWDGE
    descriptor generation for the DMA-CCE accumulate adds (u = pivot+x,
    vt = t+v).
  - The scalar coefficient chain for chunk i+1 runs during chunk i's vector
    stage (software pipelining over the cross-engine chain latency).
"""

import math

import numpy as np

import concourse.bacc as bacc
import concourse.mybir as mybir
import concourse.tile as tile
from concourse.alu_op_type import AluOpType
from concourse.bass_utils import run_bass_kernel_spmd

AFT = mybir.ActivationFunctionType
F32 = mybir.dt.float32
F16 = mybir.dt.float16

N_TOTAL = 4194304
NCORES = 8
NPC = N_TOTAL // NCORES  # 524288 points per core
P = 128
F_DEF = 512  # points per partition per chunk
EPS = 1e-6
SQRT2 = math.sqrt(2.0)

# net plane layout (host-built):
#   w0 w1 w2 w0 w1 | v0 v1 v2 | p0 p1 p2 | t0 t1 t2 | x0 x1 x2
NPLANES = 17


def build_nc(npc: int = NPC, f: int = F_DEF):
    nchunks = npc // (P * f)
    assert nchunks * P * f == npc

    nc = bacc.Bacc("TRN2", target_bir_lowering=False, debug=False)

    net = nc.dram_tensor("net", [NPLANES, npc], F16, kind="ExternalInput")
    eye = nc.dram_tensor("eye", [P, 2 * P], F16, kind="ExternalInput")  # [I | -I]
    out = nc.dram_tensor("out", [3, npc], F16, kind="ExternalOutput")

    net_r = net.ap().rearrange("c (n p f) -> n p c f", p=P, f=f)
    out_r = out.ap().rearrange("c (n p f) -> n p c f", p=P, f=f)

    V = nc.vector
    G = nc.gpsimd
    S = nc.scalar
    T = nc.tensor
    mul, add, sub = AluOpType.mult, AluOpType.add, AluOpType.subtract

    eps2 = nc.alloc_sbuf_tensor("eps2_const", [P, 1], F32)
    nc.gpsimd.memset(eps2.ap(), EPS * EPS)
    eye_sb = nc.alloc_sbuf_tensor("eye_sb", [P, 2 * P], F16)
    eye_sem = nc.alloc_semaphore("eye_sem")
    nc.sync.dma_start(out=eye_sb.ap(), in_=eye.ap()).then_inc(eye_sem, 16)
    nc.sync.wait_ge(eye_sem, 16)
    nc.all_engine_barrier()
    eye_pos = eye_sb.ap()[:, 0:P]
    eye_neg = eye_sb.ap()[:, P : 2 * P]

    with tile.TileContext(nc) as tc:
        with (
            tc.tile_pool(name="io", bufs=2) as io,
            tc.tile_pool(name="vec", bufs=2) as vec,
            tc.tile_pool(name="sc", bufs=2) as sc,
            tc.tile_pool(name="ps", bufs=1, space="PSUM") as ps,
        ):
            st: dict[int, dict] = {}  # per-chunk tiles

            def v3(ap_flat, c=3):
                return ap_flat.rearrange("p (c f) -> p c f", c=c)

            def bc3(s_ap):
                # [P,F] -> [P,3,F] plane-broadcast (step-0 middle dim)
                return s_ap.unsqueeze(1).to_broadcast((P, 3, f))

            def mm_terms(psum3, terms, first, last):
                """psum3[P, 3f] += sum of +/- terms (one matmul per plane).

                first/last flag whether these terms open/close each PSUM
                bank's accumulation group.  Emitted term-major so the PE can
                start as soon as each addend tile is produced.
                """
                for ti, (ap_flat, sign) in enumerate(terms):
                    for c in range(3):
                        T.matmul(
                            psum3[:, c * f : (c + 1) * f],
                            eye_pos if sign > 0 else eye_neg,
                            ap_flat[:, c * f : (c + 1) * f],
                            start=(first and ti == 0),
                            stop=(last and ti == len(terms) - 1),
                        )

            def issue_loads(k):
                net16 = io.tile([P, NPLANES * f], F16, tag="net", name="net16", bufs=3)
                nc.sync.dma_start(out=v3(net16[:], c=NPLANES), in_=net_r[k])
                st[k] = {"net": net16}

            def gp_accums(k):
                # u = pivot += x ; vt = t += v   (DMA CCE adds, SWDGE)
                net16 = st[k]["net"]
                G.dma_start(
                    out=net16[:, 8 * f : 11 * f],
                    in_=net16[:, 14 * f : 17 * f],
                    accum_op=add,
                )
                G.dma_start(
                    out=net16[:, 11 * f : 14 * f],
                    in_=net16[:, 5 * f : 8 * f],
                    accum_op=add,
                )

            def stile(k, tag, dt=F32):
                t = sc.tile([P, f], dt, tag=tag, name=tag + "_t")
                st[k][tag] = t
                return t

            # ---- chain stage pieces for chunk k (split across emit points) --
            def chain_act_sq(k):
                sq16 = vec.tile([P, 3 * f], F16, tag="sq", name="sq16")
                st[k]["sq"] = sq16
                S.activation(sq16[:], st[k]["net"][:, 0 : 3 * f], AFT.Square)

            def chain_dve_n2(k):
                sq16 = st[k]["sq"]
                n2h = stile(k, "n2", F16)
                V.tensor_tensor(n2h[:], sq16[:, 0:f], sq16[:, f : 2 * f], add)
                V.tensor_tensor(n2h[:], n2h[:], sq16[:, 2 * f : 3 * f], add)

            def chain_act_sqrt(k):
                th = stile(k, "th")
                # theta = sqrt(n2 + eps^2)  (~= |w| + eps as theta -> 0)
                S.activation(th[:], st[k]["n2"][:], AFT.Sqrt, bias=eps2.ap())

            def chain_dve_recip(k):
                inv = stile(k, "inv")
                thw = stile(k, "thw")
                V.reciprocal_approx_fast(out=inv[:], in_=st[k]["th"][:])
                V.add_range_wrap(thw[:], st[k]["th"][:], 0.0, math.pi, 2 * math.pi)

            def chain_act_sin(k):
                s32 = stile(k, "s32")
                sh16 = stile(k, "sh", F16)
                S.activation(s32[:], st[k]["thw"][:], AFT.Sin)
                S.activation(sh16[:], st[k]["thw"][:], AFT.Sin, scale=0.5)

            def chain_act_sq2(k):
                inv2 = stile(k, "inv2", F16)
                c116 = stile(k, "c1", F16)
                S.activation(inv2[:], st[k]["inv"][:], AFT.Square)
                # 1-cos = 2 sin(t/2)^2 = (sqrt2*sh)^2
                S.activation(c116[:], st[k]["sh"][:], AFT.Square, scale=SQRT2)

            def chain_gp_tail(k):
                thms = stile(k, "thms")
                inv3 = stile(k, "inv3")
                sg = stile(k, "sg", F16)
                G.tensor_tensor(thms[:], st[k]["th"][:], st[k]["s32"][:], sub)
                G.tensor_tensor(inv3[:], st[k]["inv2"][:], st[k]["inv"][:], mul)
                G.tensor_tensor(sg[:], thms[:], inv3[:], mul)

            def chain_dve_k(k):
                k1 = stile(k, "k1", F16)
                k2 = stile(k, "k2", F16)
                V.tensor_tensor(k1[:], st[k]["s32"][:], st[k]["inv"][:], mul)
                V.tensor_tensor(k2[:], st[k]["c1"][:], st[k]["inv2"][:], mul)

            # ---------------- prologue ----------------
            issue_loads(0)
            if nchunks > 1:
                issue_loads(1)
            gp_accums(0)
            chain_act_sq(0)
            chain_dve_n2(0)
            chain_act_sqrt(0)
            chain_dve_recip(0)
            chain_act_sin(0)
            chain_act_sq2(0)
            chain_gp_tail(0)
            chain_dve_k(0)

            defer = None  # (o16, k) pending store

            for i in range(nchunks):
                d = st[i]
                net16 = d["net"]
                w_ext = net16[:, 0 : 5 * f]
                v3f = net16[:, 5 * f : 8 * f]
                u3f = net16[:, 8 * f : 11 * f]
                vt3f = net16[:, 11 * f : 14 * f]
                k1, k2, sg = d["k1"], d["k2"], d["sg"]

                m_gu = vec.tile([P, 3 * f], F16, tag="gu", bufs=3, name="m_gu")
                m_gv = vec.tile([P, 3 * f], F16, tag="gv", bufs=3, name="m_gv")
                h_ext = vec.tile([P, 5 * f], F16, tag="h", name="h_ext")
                m_hv = vec.tile([P, 3 * f], F16, tag="hv", bufs=3, name="m_hv")
                cr1 = vec.tile([P, 3 * f], F16, tag="cr1", bufs=3, name="cr1")
                m_c1b = vec.tile([P, 3 * f], F16, tag="cr1b", bufs=3, name="m_c1b")
                s_ext = vec.tile([P, 5 * f], F16, tag="s", name="s_ext")
                m_c2a = vec.tile([P, 3 * f], F16, tag="cr2a", bufs=3, name="m_c2a")
                m_c2b = vec.tile([P, 3 * f], F16, tag="cr2b", bufs=3, name="m_c2b")
                # two PSUM regions per iteration from one double-buffered tag:
                # s always lands in slot 0, o in slot 1
                p_s = ps.tile([P, 3 * f], F32, tag="psx", name="p_s", bufs=2)
                p_o = ps.tile([P, 3 * f], F32, tag="psx", name="p_o", bufs=2)
                o16 = io.tile([P, 3 * f], F16, tag="o", name="o16", bufs=2)
                h3 = h_ext[:, 0 : 3 * f]

                if i + 2 < nchunks:
                    issue_loads(i + 2)

                # --- DVE vector stage for chunk i, chain(i+1) + PE terms
                #     slotted in as their inputs are produced ---
                V.tensor_tensor(v3(m_gu[:]), v3(u3f), bc3(k1[:]), mul)  # gu
                V.tensor_tensor(v3(m_gv[:]), v3(v3f), bc3(k2[:]), mul)  # gv
                mm_terms(p_s[:], [(m_gu[:], 1), (m_gv[:], 1)], first=True, last=False)
                mm_terms(p_o[:], [(vt3f, 1)], first=True, last=False)
                if i + 1 < nchunks:
                    chain_act_sq(i + 1)  # ACT
                V.tensor_tensor(v3(h3), v3(u3f), bc3(k2[:]), mul)  # hu
                V.tensor_tensor(v3(m_hv[:]), v3(v3f), bc3(sg[:]), mul)  # hv
                V.tensor_tensor(h3, h3, m_hv[:], add)  # h = hu + hv
                V.tensor_copy(h_ext[:, 3 * f : 5 * f], h_ext[:, 0 : 2 * f])
                if i + 1 < nchunks:
                    chain_dve_n2(i + 1)  # DVE
                    chain_act_sqrt(i + 1)  # ACT
                V.tensor_tensor(
                    cr1[:], w_ext[:, f : 4 * f], h_ext[:, 2 * f : 5 * f], mul
                )
                mm_terms(p_s[:], [(cr1[:], 1)], first=False, last=False)
                V.tensor_tensor(
                    m_c1b[:], w_ext[:, 2 * f : 5 * f], h_ext[:, f : 4 * f], mul
                )
                mm_terms(p_s[:], [(m_c1b[:], -1)], first=False, last=True)
                if i + 1 < nchunks:
                    chain_dve_recip(i + 1)  # DVE customs
                    chain_act_sin(i + 1)  # ACT
                S.activation(s_ext[:, 0 : 3 * f], p_s[:], AFT.Copy)  # PSUM -> SBUF
                if i + 1 < nchunks:
                    chain_act_sq2(i + 1)  # ACT
                    chain_gp_tail(i + 1)  # GPSIMD
                    chain_dve_k(i + 1)  # DVE
                V.tensor_copy(s_ext[:, 3 * f : 5 * f], s_ext[:, 0 : 2 * f])
                V.tensor_tensor(
                    m_c2a[:], w_ext[:, f : 4 * f], s_ext[:, 2 * f : 5 * f], mul
                )
                mm_terms(p_o[:], [(m_c2a[:], 1)], first=False, last=False)
                V.tensor_tensor(
                    m_c2b[:], w_ext[:, 2 * f : 5 * f], s_ext[:, f : 4 * f], mul
                )
                mm_terms(p_o[:], [(m_c2b[:], -1)], first=False, last=True)
                S.activation(o16[:], p_o[:], AFT.Copy)  # PSUM -> SBUF f16
                if i + 1 < nchunks:
                    gp_accums(i + 1)

                if defer is not None:
                    o_prev, k_prev = defer
                    nc.sync.dma_start(out=out_r[k_prev], in_=v3(o_prev[:]))
                defer = (o16, i)

            o_prev, k_prev = defer
            nc.sync.dma_start(out=out_r[k_prev], in_=v3(o_prev[:]))

    nc.compile()
    return nc


_NC_CACHE: dict = {}


def _get_nc():
    if "nc" not in _NC_CACHE:
        _NC_CACHE["nc"] = build_nc()
    return _NC_CACHE["nc"]


def _make_eye() -> np.ndarray:
    i = np.eye(P, dtype=np.float16)
    return np.concatenate([i, -i], axis=1)


def make_in_maps(pos: np.ndarray, net: np.ndarray, npc: int = NPC, ncores: int = NCORES):
    """Build per-core planar fp16 input maps (net has w0,w1 duplicated)."""
    eye = _make_eye()
    in_maps = []
    for i in range(ncores):
        sl = slice(i * npc, (i + 1) * npc)
        net_c = net[sl].astype(np.float16)  # [npc, 12]
        pos_c = pos[sl].astype(np.float16)  # [npc, 3]
        net_ext = np.empty((NPLANES, npc), np.float16)
        for dst, col in enumerate([0, 1, 2, 0, 1, 3, 4, 5, 6, 7, 8, 9, 10, 11]):
            net_ext[dst] = net_c[:, col]
        for c in range(3):
            net_ext[14 + c] = pos_c[:, c]
        in_maps.append({"net": net_ext, "eye": eye})
    return in_maps


def kernel(undeformed_positions: np.ndarray, network_output: np.ndarray) -> np.ndarray:
    pos = np.asarray(undeformed_positions, dtype=np.float32)
    net = np.asarray(network_output, dtype=np.float32)
    assert pos.shape == (N_TOTAL, 3) and net.shape == (N_TOTAL, 12)

    nc = _get_nc()
    in_maps = make_in_maps(pos, net)
    res = run_bass_kernel_spmd(nc, in_maps, list(range(NCORES)))
    out = np.empty((N_TOTAL, 3), np.float32)
    for i in range(NCORES):
        o = res.results[i]["out"]  # [3, npc] f16
        out[i * NPC : (i + 1) * NPC] = o.T.astype(np.float32)
    return out
